# revision 28
# baseline (speedup 1.0000x reference)
"""Multi-head self-attention (B=2, T=2048, E=1024, H=16) on 8 TRN2 NeuronCores.

Sharding: tensor-parallel over heads (2 heads per core) for QKV + attention,
then an AllToAll reshards to token-parallel (512 tokens per core) for the
output projection. The final gather/transpose happens on host.

Layout choices (no on-device transposes anywhere):
  - Host passes x transposed: xT [E, B*T] bf16 (features on partitions).
  - QKV computed as Q.T/K.T [128feat, 4096tok] (weight-stationary matmuls)
    and V as [tok, feat] (x-stationary matmuls), with a ones column appended
    (V') so that exp(S).T @ V' also yields the softmax denominator Z.
  - Scores computed transposed S.T[k, q] = K @ Q.T so the softmax reduction
    is a by-product of the attention matmul; exp() runs on ACT from PSUM.
    No max-subtraction: inputs are well-scaled, |S*0.125| < ~6.
  - Normalization 1/Z broadcast across the D partitions via a K=1
    ones-matmul, staged through SBUF for the DVE multiply.

The softmax exp on ACT (1 elem/lane/cycle) and the PE matmuls are the two
near-critical engines; the head-step loop is software-pipelined (scores+exp
of step i overlap attention@V of step i-1), the all-to-all is split per
head so the first half hides under compute, and the output projection runs
in two half-K passes so its first half hides under the second all-to-all.
"""

import numpy as np
import ml_dtypes

import concourse.bass as bass
import concourse.mybir as mybir
import concourse.tile as tile
from concourse import bacc, bass_utils

B, T, E, H, D = 2, 2048, 1024, 16, 64
NCORES = 8
HPC = H // NCORES            # heads per core = 2
FPC = HPC * D                # features per core = 128
TOK = B * T                  # 4096 global tokens
TSL = TOK // NCORES          # token slice per core = 512
QC = 1024                    # q chunk width in attention
NKT = T // 128               # k tiles per batch = 16
NIF = E // 128               # input-feature tiles = 8
XC = 1024                    # token chunk width in QKV phase
SCALE = 1.0 / float(np.sqrt(D))

F32 = mybir.dt.float32
BF16 = mybir.dt.bfloat16
_BF = ml_dtypes.bfloat16


def build_nc():
    nc = bacc.Bacc(None, target_bir_lowering=False, num_devices=NCORES)

    xT = nc.declare_dram_parameter("xT", [E, TOK], BF16, isOutput=False)
    wqT = nc.declare_dram_parameter("wqT", [E, FPC], BF16, isOutput=False)
    wkT = nc.declare_dram_parameter("wkT", [E, FPC], BF16, isOutput=False)
    wvT = nc.declare_dram_parameter("wvT", [E, FPC], BF16, isOutput=False)
    bq = nc.declare_dram_parameter("bq", [FPC, 1], F32, isOutput=False)
    bk = nc.declare_dram_parameter("bk", [FPC, 1], F32, isOutput=False)
    bv = nc.declare_dram_parameter("bv", [FPC, 1], F32, isOutput=False)
    owT = nc.declare_dram_parameter("owT", [E, E], BF16, isOutput=False)
    ob = nc.declare_dram_parameter("ob", [128, NIF], F32, isOutput=False)
    out = nc.declare_dram_parameter("out", [E, TSL], F32, isOutput=True)

    # Per-head-half collective bounce buffers: the head-A all-to-all fires
    # while head-B attention still computes.
    cc_in = [nc.dram_tensor(f"cc_in{h}", [NCORES, D, TSL], BF16) for h in range(HPC)]
    cc_out = [nc.dram_tensor(f"cc_out{h}", [NCORES, D, TSL], BF16) for h in range(HPC)]

    NTT = TOK // 128  # 32 token tiles for V'

    def mm_wide(out_ap, lhsT, rhs, start, stop, n=512):
        """Matmul with wide N: split into bank-sized column chunks (fp32 out
        must not cross a PSUM bank boundary)."""
        N = rhs.shape[-1]
        for c0 in range(0, N, n):
            cs = slice(c0, min(c0 + n, N))
            nc.tensor.matmul(
                out_ap[:, cs], lhsT=lhsT, rhs=rhs[:, cs], start=start, stop=stop
            )

    with tile.TileContext(nc) as tc:
        with tc.tile_pool(name="const", bufs=1) as cpool:
            # ---- persistent SBUF tensors ------------------------------
            wq_sb = cpool.tile([128, NIF, FPC], BF16, tag="wq")
            wk_sb = cpool.tile([128, NIF, FPC], BF16, tag="wk")
            wv_sb = cpool.tile([128, NIF, FPC], BF16, tag="wv")
            bq_sb = cpool.tile([FPC, 1], F32, tag="bq")
            bk_sb = cpool.tile([FPC, 1], F32, tag="bk")
            bv_sb = cpool.tile([FPC, 1], F32, tag="bv")
            nc.sync.dma_start(out=bq_sb, in_=bq[:, :])
            nc.sync.dma_start(out=bk_sb, in_=bk[:, :])
            nc.sync.dma_start(out=bv_sb, in_=bv[:, :])
            nc.sync.dma_start(out=wq_sb, in_=wqT.rearrange("(i p) f -> p i f", p=128))
            nc.sync.dma_start(out=wk_sb, in_=wkT.rearrange("(i p) f -> p i f", p=128))
            nc.sync.dma_start(out=wv_sb, in_=wvT.rearrange("(i p) f -> p i f", p=128))

            # out-proj weights arrive on the gpsimd queue so they don't
            # delay the x loads; only needed in phase 3.
            ow_sb = cpool.tile([128, NIF, NIF, 128], BF16, tag="ow")
            ob_sb = cpool.tile([128, NIF], F32, tag="ob")
            nc.gpsimd.dma_start(out=ob_sb, in_=ob[:, :])
            nc.gpsimd.dma_start(
                out=ow_sb,
                in_=owT.rearrange("(i p) (o f) -> p i o f", p=128, f=128),
            )

            ones_sb = cpool.tile([1, D], BF16, tag="ones")
            nc.vector.memset(ones_sb, 1.0)

            qT_sb = cpool.tile([128, TOK], BF16, tag="qT")
            kT_sb = cpool.tile([128, TOK], BF16, tag="kT")
            # V' layout per 128-token tile: [64 headA | one | 64 headB | one]
            vp_sb = cpool.tile([128, NTT, 2 * (D + 1)], BF16, tag="vp")
            nc.vector.memset(vp_sb[:, :, D : D + 1], 1.0)
            nc.vector.memset(vp_sb[:, :, 2 * D + 1 : 2 * D + 2], 1.0)

            yT_sb = cpool.tile([128, TOK], BF16, tag="yT")
            ytf_sb = cpool.tile([128, NCORES, TSL], BF16, tag="ytf")

            # ---- phase 1: QKV projections -----------------------------
            with (
                tc.tile_pool(name="xt", bufs=2) as xpool,
                tc.tile_pool(name="ps1", bufs=3, space="PSUM") as ps1,
            ):
                for tcx in range(TOK // XC):
                    tsl = bass.ts(tcx, XC)
                    xt = xpool.tile([128, NIF, XC], BF16)
                    for i in range(NIF):
                        nc.sync.dma_start(
                            out=xt[:, i, :],
                            in_=xT[i * 128 : (i + 1) * 128, tsl],
                        )
                    # Q.T and K.T : weight-stationary
                    for w_sb, b_sb, dst in (
                        (wq_sb, bq_sb, qT_sb),
                        (wk_sb, bk_sb, kT_sb),
                    ):
                        ps = ps1.tile([128, XC], F32, tag="ps")
                        for i in range(NIF):
                            mm_wide(
                                ps,
                                lhsT=w_sb[:, i, :],
                                rhs=xt[:, i, :],
                                start=(i == 0),
                                stop=(i == NIF - 1),
                            )
                        nc.vector.tensor_scalar_add(dst[:, tsl], ps, b_sb)
                    # V : x-stationary, [tok, feat] layout into V'
                    for tt8 in range(XC // 128):
                        psv = ps1.tile([128, FPC], F32, tag="ps")
                        for i in range(NIF):
                            nc.tensor.matmul(
                                psv,
                                lhsT=xt[:, i, bass.ts(tt8, 128)],
                                rhs=wv_sb[:, i, :],
                                start=(i == 0),
                                stop=(i == NIF - 1),
                            )
                        tt = tcx * (XC // 128) + tt8
                        nc.vector.tensor_copy(vp_sb[:, tt, 0:D], psv[:, 0:D])
                        nc.vector.tensor_copy(
                            vp_sb[:, tt, D + 1 : 2 * D + 1], psv[:, D : 2 * D]
                        )

            # ---- phase 2: attention, software-pipelined head-steps ----
            # Step order is head-major so head-A's all-to-all fires while
            # head-B attention still computes.
            with (
                tc.tile_pool(name="pss", bufs=2, space="PSUM") as psspool,
                tc.tile_pool(name="psav", bufs=2, space="PSUM") as avpool,
                tc.tile_pool(name="exp", bufs=2) as epool,
                tc.tile_pool(name="small", bufs=4) as spool,
            ):
                steps = [
                    (h, b, q2)
                    for h in range(HPC)
                    for b in range(B)
                    for q2 in range(T // QC)
                ]

                def scores_exp(step):
                    h, b, q2 = step
                    hsl = slice(h * D, (h + 1) * D)
                    qsl = bass.ds(b * T + q2 * QC, QC)
                    ex = epool.tile([128, NKT, QC], BF16, tag="ex")
                    for kt in range(NKT):
                        pss = psspool.tile([128, QC], F32, tag="pss")
                        mm_wide(
                            pss,
                            lhsT=kT_sb[hsl, bass.ds(b * T + kt * 128, 128)],
                            rhs=qT_sb[hsl, qsl],
                            start=True,
                            stop=True,
                        )
                        nc.scalar.activation(
                            out=ex[:, kt, :],
                            in_=pss,
                            func=mybir.ActivationFunctionType.Exp,
                            scale=float(SCALE),
                        )
                    return ex

                def av_norm(step, ex):
                    h, b, q2 = step
                    hsl = slice(h * D, (h + 1) * D)
                    vsl = slice(h * (D + 1), (h + 1) * (D + 1))
                    qsl = bass.ds(b * T + q2 * QC, QC)
                    psav = avpool.tile([D + 1, QC], F32, tag="av")
                    for kt in range(NKT):
                        mm_wide(
                            psav,
                            lhsT=vp_sb[:, b * NKT + kt, vsl],
                            rhs=ex[:, kt, :],
                            start=(kt == 0),
                            stop=(kt == NKT - 1),
                        )
                    # r = 1/Z, broadcast across the D partitions via a K=1
                    # ones-matmul, staged through SBUF for the DVE multiply.
                    r = spool.tile([1, QC], BF16, tag="r")
                    with nc.allow_low_precision(reason="1/Z in bf16"):
                        nc.vector.reciprocal(r, psav[D : D + 1, :])
                    psb = avpool.tile([D, QC], F32, tag="av")
                    mm_wide(psb, lhsT=ones_sb, rhs=r, start=True, stop=True)
                    r_bc = spool.tile([D, QC], F32, tag="rbc")
                    nc.vector.tensor_copy(r_bc, psb)
                    ysl = yT_sb[hsl, qsl]
                    nc.vector.tensor_mul(ysl, psav[0:D, :], r_bc)
                    nc.vector.tensor_scalar_add(ysl, ysl, bv_sb[hsl, :])
                    # ship this step's two token-shards to the collective
                    # bounce buffer as soon as they're final
                    for j in (2 * (b * 2 + q2), 2 * (b * 2 + q2) + 1):
                        nc.sync.dma_start(
                            out=cc_in[h][j],
                            in_=yT_sb[hsl, bass.ts(j, TSL)],
                        )

                def a2a(h):
                    nc.gpsimd.collective_compute(
                        "AllToAll",
                        mybir.AluOpType.bypass,
                        replica_groups=[list(range(NCORES))],
                        ins=[cc_in[h][:].opt()],
                        outs=[cc_out[h][:].opt()],
                    )
                    rs = slice(h * D, (h + 1) * D)
                    for i in range(NCORES):
                        nc.sync.dma_start(out=ytf_sb[rs, i, :], in_=cc_out[h][i])

                half = len(steps) // 2  # steps [0:half] are head A
                prev = None
                for si, step in enumerate(steps):
                    ex = scores_exp(step)
                    if prev is not None:
                        av_norm(*prev)
                    if si == half:
                        a2a(0)  # all head-A av_norms have been emitted
                    prev = (step, ex)
                av_norm(*prev)
                a2a(1)

            # ---- phase 3: output projection (two half-K passes so the
            # head-A half overlaps the head-B all-to-all) ----------------
            with (
                tc.tile_pool(name="ps3", bufs=1, space="PSUM") as ps3,
                tc.tile_pool(name="outp", bufs=3) as opool,
            ):
                psos = []
                for o in range(NIF):
                    pso = ps3.tile([128, TSL], F32, tag=f"po{o}")
                    for i in range(NIF):
                        nc.tensor.matmul(
                            pso,
                            lhsT=ow_sb[0:D, i, o, :],
                            rhs=ytf_sb[0:D, i, :],
                            start=(i == 0),
                            stop=False,
                        )
                    psos.append(pso)
                for o in range(NIF):
                    pso = psos[o]
                    for i in range(NIF):
                        nc.tensor.matmul(
                            pso,
                            lhsT=ow_sb[D:128, i, o, :],
                            rhs=ytf_sb[D:128, i, :],
                            start=False,
                            stop=(i == NIF - 1),
                        )
                    osb = opool.tile([128, TSL], F32)
                    nc.vector.tensor_scalar_add(osb, pso, ob_sb[:, o : o + 1])
                    nc.sync.dma_start(out=out[o * 128 : (o + 1) * 128, :], in_=osb)

    nc.finalize()
    return nc


def make_in_maps(x, qkv_w, qkv_b, out_w, out_b):
    x = np.asarray(x, dtype=np.float32).reshape(TOK, E)
    qkv_w = np.asarray(qkv_w, dtype=np.float32)
    qkv_b = np.asarray(qkv_b, dtype=np.float32)
    out_w = np.asarray(out_w, dtype=np.float32)
    out_b = np.asarray(out_b, dtype=np.float32)

    xT = np.ascontiguousarray(x.T).astype(_BF)
    owT = np.ascontiguousarray(out_w.T).astype(_BF)
    ob = np.ascontiguousarray(out_b.reshape(NIF, 128).T.astype(np.float32))

    in_maps = []
    for c in range(NCORES):
        rs = slice(c * FPC, (c + 1) * FPC)
        ks = slice(E + c * FPC, E + (c + 1) * FPC)
        vs = slice(2 * E + c * FPC, 2 * E + (c + 1) * FPC)
        in_maps.append(
            {
                "xT": xT,
                "wqT": np.ascontiguousarray(qkv_w[rs, :].T).astype(_BF),
                "wkT": np.ascontiguousarray(qkv_w[ks, :].T).astype(_BF),
                "wvT": np.ascontiguousarray(qkv_w[vs, :].T).astype(_BF),
                "bq": qkv_b[rs].reshape(FPC, 1).copy(),
                "bk": qkv_b[ks].reshape(FPC, 1).copy(),
                "bv": qkv_b[vs].reshape(FPC, 1).copy(),
                "owT": owT,
                "ob": ob,
            }
        )
    return in_maps


def assemble(results):
    full = np.empty((TOK, E), dtype=np.float32)
    for c in range(NCORES):
        full[c * TSL : (c + 1) * TSL, :] = results[c]["out"].T
    return full.reshape(B, T, E)


_NC_CACHE = None


def kernel(x, qkv_w, qkv_b, out_w, out_b):
    global _NC_CACHE
    if _NC_CACHE is None:
        _NC_CACHE = build_nc()
    in_maps = make_in_maps(x, qkv_w, qkv_b, out_w, out_b)
    res = bass_utils.run_bass_kernel_spmd(
        _NC_CACHE, in_maps, core_ids=list(range(NCORES))
    )
    return assemble(res.results)


# revision 29
# speedup vs baseline: 1.0111x; 1.0111x over previous
"""Multi-head self-attention (B=2, T=2048, E=1024, H=16) on 8 TRN2 NeuronCores.

Sharding: tensor-parallel over heads (2 heads per core) for QKV + attention,
then an AllToAll reshards to token-parallel (512 tokens per core) for the
output projection. The final gather/transpose happens on host.

Layout choices (no on-device transposes anywhere):
  - Host passes x transposed: xT [E, B*T] bf16 (features on partitions).
  - QKV computed as Q.T/K.T [128feat, 4096tok] (weight-stationary matmuls)
    and V as [tok, feat] (x-stationary matmuls), with a ones column appended
    (V') so that exp(S).T @ V' also yields the softmax denominator Z.
  - Scores computed transposed S.T[k, q] = K @ Q.T so the softmax reduction
    is a by-product of the attention matmul; exp() runs on ACT from PSUM.
    No max-subtraction: inputs are well-scaled, |S*0.125| < ~6.
  - Normalization 1/Z broadcast across the D partitions via a K=1
    ones-matmul, staged through SBUF for the DVE multiply.

The softmax exp on ACT (1 elem/lane/cycle) and the PE matmuls are the two
near-critical engines; the head-step loop is software-pipelined (scores+exp
of step i overlap attention@V of step i-1), the all-to-all is split per
head so the first half hides under compute, and the output projection runs
in two half-K passes so its first half hides under the second all-to-all.
"""

import numpy as np
import ml_dtypes

import concourse.bass as bass
import concourse.mybir as mybir
import concourse.tile as tile
from concourse import bacc, bass_utils

B, T, E, H, D = 2, 2048, 1024, 16, 64
NCORES = 8
HPC = H // NCORES            # heads per core = 2
FPC = HPC * D                # features per core = 128
TOK = B * T                  # 4096 global tokens
TSL = TOK // NCORES          # token slice per core = 512
QC = 1024                    # q chunk width in attention
NKT = T // 128               # k tiles per batch = 16
NIF = E // 128               # input-feature tiles = 8
XC = 1024                    # token chunk width in QKV phase
SCALE = 1.0 / float(np.sqrt(D))

F32 = mybir.dt.float32
BF16 = mybir.dt.bfloat16
_BF = ml_dtypes.bfloat16


def build_nc():
    nc = bacc.Bacc(None, target_bir_lowering=False, num_devices=NCORES)

    xT = nc.declare_dram_parameter("xT", [E, TOK], BF16, isOutput=False)
    wqT = nc.declare_dram_parameter("wqT", [E, FPC], BF16, isOutput=False)
    wkT = nc.declare_dram_parameter("wkT", [E, FPC], BF16, isOutput=False)
    wvT = nc.declare_dram_parameter("wvT", [E, FPC], BF16, isOutput=False)
    bq = nc.declare_dram_parameter("bq", [FPC, 1], F32, isOutput=False)
    bk = nc.declare_dram_parameter("bk", [FPC, 1], F32, isOutput=False)
    bv = nc.declare_dram_parameter("bv", [FPC, 1], F32, isOutput=False)
    owT = nc.declare_dram_parameter("owT", [E, E], BF16, isOutput=False)
    ob = nc.declare_dram_parameter("ob", [128, NIF], F32, isOutput=False)
    out = nc.declare_dram_parameter("out", [E, TSL], F32, isOutput=True)

    # Per-head-half collective bounce buffers: the head-A all-to-all fires
    # while head-B attention still computes.
    cc_in = [nc.dram_tensor(f"cc_in{h}", [NCORES, D, TSL], BF16) for h in range(HPC)]
    cc_out = [nc.dram_tensor(f"cc_out{h}", [NCORES, D, TSL], BF16) for h in range(HPC)]

    NTT = TOK // 128  # 32 token tiles for V'

    def mm_wide(out_ap, lhsT, rhs, start, stop, n=512):
        """Matmul with wide N: split into bank-sized column chunks (fp32 out
        must not cross a PSUM bank boundary)."""
        N = rhs.shape[-1]
        for c0 in range(0, N, n):
            cs = slice(c0, min(c0 + n, N))
            nc.tensor.matmul(
                out_ap[:, cs], lhsT=lhsT, rhs=rhs[:, cs], start=start, stop=stop
            )

    with tile.TileContext(nc) as tc:
        with tc.tile_pool(name="const", bufs=1) as cpool:
            # ---- persistent SBUF tensors ------------------------------
            wq_sb = cpool.tile([128, NIF, FPC], BF16, tag="wq")
            wk_sb = cpool.tile([128, NIF, FPC], BF16, tag="wk")
            wv_sb = cpool.tile([128, NIF, FPC], BF16, tag="wv")
            bq_sb = cpool.tile([FPC, 1], F32, tag="bq")
            bk_sb = cpool.tile([FPC, 1], F32, tag="bk")
            bv_sb = cpool.tile([FPC, 1], F32, tag="bv")
            nc.sync.dma_start(out=bq_sb, in_=bq[:, :])
            nc.sync.dma_start(out=bk_sb, in_=bk[:, :])
            nc.sync.dma_start(out=bv_sb, in_=bv[:, :])
            nc.sync.dma_start(out=wq_sb, in_=wqT.rearrange("(i p) f -> p i f", p=128))
            nc.sync.dma_start(out=wk_sb, in_=wkT.rearrange("(i p) f -> p i f", p=128))
            nc.sync.dma_start(out=wv_sb, in_=wvT.rearrange("(i p) f -> p i f", p=128))

            # out-proj weights arrive on the gpsimd queue so they don't
            # delay the x loads; only needed in phase 3.
            ow_sb = cpool.tile([128, NIF, NIF, 128], BF16, tag="ow")
            ob_sb = cpool.tile([128, NIF], F32, tag="ob")
            nc.gpsimd.dma_start(out=ob_sb, in_=ob[:, :])
            nc.gpsimd.dma_start(
                out=ow_sb,
                in_=owT.rearrange("(i p) (o f) -> p i o f", p=128, f=128),
            )

            ones_sb = cpool.tile([1, D], BF16, tag="ones")
            nc.vector.memset(ones_sb, 1.0)

            qT_sb = cpool.tile([128, TOK], BF16, tag="qT")
            kT_sb = cpool.tile([128, TOK], BF16, tag="kT")
            # V' layout per 128-token tile: [64 headA | one | 64 headB | one]
            vp_sb = cpool.tile([128, NTT, 2 * (D + 1)], BF16, tag="vp")
            nc.vector.memset(vp_sb[:, :, D : D + 1], 1.0)
            nc.vector.memset(vp_sb[:, :, 2 * D + 1 : 2 * D + 2], 1.0)

            yT_sb = cpool.tile([128, TOK], BF16, tag="yT")
            ytf_sb = cpool.tile([128, NCORES, TSL], BF16, tag="ytf")

            # ---- phase 1: QKV projections -----------------------------
            with (
                tc.tile_pool(name="xt", bufs=3) as xpool,
                tc.tile_pool(name="ps1", bufs=4, space="PSUM") as ps1,
            ):
                for tcx in range(TOK // XC):
                    tsl = bass.ts(tcx, XC)
                    xt = xpool.tile([128, NIF, XC], BF16)
                    for i in range(NIF):
                        nc.sync.dma_start(
                            out=xt[:, i, :],
                            in_=xT[i * 128 : (i + 1) * 128, tsl],
                        )
                    # Q.T and K.T : weight-stationary
                    for w_sb, b_sb, dst in (
                        (wq_sb, bq_sb, qT_sb),
                        (wk_sb, bk_sb, kT_sb),
                    ):
                        ps = ps1.tile([128, XC], F32, tag="ps")
                        for i in range(NIF):
                            mm_wide(
                                ps,
                                lhsT=w_sb[:, i, :],
                                rhs=xt[:, i, :],
                                start=(i == 0),
                                stop=(i == NIF - 1),
                            )
                        nc.vector.tensor_scalar_add(dst[:, tsl], ps, b_sb)
                    # V : x-stationary, [tok, feat] layout into V'
                    for tt8 in range(XC // 128):
                        psv = ps1.tile([128, FPC], F32, tag="ps")
                        for i in range(NIF):
                            nc.tensor.matmul(
                                psv,
                                lhsT=xt[:, i, bass.ts(tt8, 128)],
                                rhs=wv_sb[:, i, :],
                                start=(i == 0),
                                stop=(i == NIF - 1),
                            )
                        tt = tcx * (XC // 128) + tt8
                        nc.vector.tensor_copy(vp_sb[:, tt, 0:D], psv[:, 0:D])
                        nc.vector.tensor_copy(
                            vp_sb[:, tt, D + 1 : 2 * D + 1], psv[:, D : 2 * D]
                        )

            # ---- phase 2: attention, software-pipelined head-steps ----
            # Step order is head-major so head-A's all-to-all fires while
            # head-B attention still computes.
            with (
                tc.tile_pool(name="pss", bufs=2, space="PSUM") as psspool,
                tc.tile_pool(name="psav", bufs=2, space="PSUM") as avpool,
                tc.tile_pool(name="exp", bufs=2) as epool,
                tc.tile_pool(name="small", bufs=4) as spool,
            ):
                steps = [
                    (h, b, q2)
                    for h in range(HPC)
                    for b in range(B)
                    for q2 in range(T // QC)
                ]

                def scores_exp(step):
                    h, b, q2 = step
                    hsl = slice(h * D, (h + 1) * D)
                    qsl = bass.ds(b * T + q2 * QC, QC)
                    ex = epool.tile([128, NKT, QC], BF16, tag="ex")
                    for kt in range(NKT):
                        pss = psspool.tile([128, QC], F32, tag="pss")
                        mm_wide(
                            pss,
                            lhsT=kT_sb[hsl, bass.ds(b * T + kt * 128, 128)],
                            rhs=qT_sb[hsl, qsl],
                            start=True,
                            stop=True,
                        )
                        nc.scalar.activation(
                            out=ex[:, kt, :],
                            in_=pss,
                            func=mybir.ActivationFunctionType.Exp,
                            scale=float(SCALE),
                        )
                    return ex

                def av_norm(step, ex):
                    h, b, q2 = step
                    hsl = slice(h * D, (h + 1) * D)
                    vsl = slice(h * (D + 1), (h + 1) * (D + 1))
                    qsl = bass.ds(b * T + q2 * QC, QC)
                    psav = avpool.tile([D + 1, QC], F32, tag="av")
                    for kt in range(NKT):
                        mm_wide(
                            psav,
                            lhsT=vp_sb[:, b * NKT + kt, vsl],
                            rhs=ex[:, kt, :],
                            start=(kt == 0),
                            stop=(kt == NKT - 1),
                        )
                    # r = 1/Z, broadcast across the D partitions via a K=1
                    # ones-matmul, staged through SBUF for the DVE multiply.
                    r = spool.tile([1, QC], BF16, tag="r")
                    with nc.allow_low_precision(reason="1/Z in bf16"):
                        nc.vector.reciprocal(r, psav[D : D + 1, :])
                    psb = avpool.tile([D, QC], F32, tag="av")
                    mm_wide(psb, lhsT=ones_sb, rhs=r, start=True, stop=True)
                    r_bc = spool.tile([D, QC], F32, tag="rbc")
                    nc.vector.tensor_copy(r_bc, psb)
                    ysl = yT_sb[hsl, qsl]
                    nc.vector.tensor_mul(ysl, psav[0:D, :], r_bc)
                    nc.vector.tensor_scalar_add(ysl, ysl, bv_sb[hsl, :])
                    # ship this step's two token-shards to the collective
                    # bounce buffer as soon as they're final
                    for j in (2 * (b * 2 + q2), 2 * (b * 2 + q2) + 1):
                        nc.sync.dma_start(
                            out=cc_in[h][j],
                            in_=yT_sb[hsl, bass.ts(j, TSL)],
                        )

                def a2a(h):
                    nc.gpsimd.collective_compute(
                        "AllToAll",
                        mybir.AluOpType.bypass,
                        replica_groups=[list(range(NCORES))],
                        ins=[cc_in[h][:].opt()],
                        outs=[cc_out[h][:].opt()],
                    )
                    rs = slice(h * D, (h + 1) * D)
                    for i in range(NCORES):
                        nc.sync.dma_start(out=ytf_sb[rs, i, :], in_=cc_out[h][i])

                half = len(steps) // 2  # steps [0:half] are head A
                prev = None
                for si, step in enumerate(steps):
                    ex = scores_exp(step)
                    if prev is not None:
                        av_norm(*prev)
                    if si == half:
                        a2a(0)  # all head-A av_norms have been emitted
                    prev = (step, ex)
                av_norm(*prev)
                a2a(1)

            # ---- phase 3: output projection (two half-K passes so the
            # head-A half overlaps the head-B all-to-all) ----------------
            with (
                tc.tile_pool(name="ps3", bufs=1, space="PSUM") as ps3,
                tc.tile_pool(name="outp", bufs=3) as opool,
            ):
                psos = []
                for o in range(NIF):
                    pso = ps3.tile([128, TSL], F32, tag=f"po{o}")
                    for i in range(NIF):
                        nc.tensor.matmul(
                            pso,
                            lhsT=ow_sb[0:D, i, o, :],
                            rhs=ytf_sb[0:D, i, :],
                            start=(i == 0),
                            stop=False,
                        )
                    psos.append(pso)
                for o in range(NIF):
                    pso = psos[o]
                    for i in range(NIF):
                        nc.tensor.matmul(
                            pso,
                            lhsT=ow_sb[D:128, i, o, :],
                            rhs=ytf_sb[D:128, i, :],
                            start=False,
                            stop=(i == NIF - 1),
                        )
                    osb = opool.tile([128, TSL], F32)
                    nc.vector.tensor_scalar_add(osb, pso, ob_sb[:, o : o + 1])
                    nc.sync.dma_start(out=out[o * 128 : (o + 1) * 128, :], in_=osb)

    nc.finalize()
    return nc


def make_in_maps(x, qkv_w, qkv_b, out_w, out_b):
    x = np.asarray(x, dtype=np.float32).reshape(TOK, E)
    qkv_w = np.asarray(qkv_w, dtype=np.float32)
    qkv_b = np.asarray(qkv_b, dtype=np.float32)
    out_w = np.asarray(out_w, dtype=np.float32)
    out_b = np.asarray(out_b, dtype=np.float32)

    xT = np.ascontiguousarray(x.T).astype(_BF)
    owT = np.ascontiguousarray(out_w.T).astype(_BF)
    ob = np.ascontiguousarray(out_b.reshape(NIF, 128).T.astype(np.float32))

    in_maps = []
    for c in range(NCORES):
        rs = slice(c * FPC, (c + 1) * FPC)
        ks = slice(E + c * FPC, E + (c + 1) * FPC)
        vs = slice(2 * E + c * FPC, 2 * E + (c + 1) * FPC)
        in_maps.append(
            {
                "xT": xT,
                "wqT": np.ascontiguousarray(qkv_w[rs, :].T).astype(_BF),
                "wkT": np.ascontiguousarray(qkv_w[ks, :].T).astype(_BF),
                "wvT": np.ascontiguousarray(qkv_w[vs, :].T).astype(_BF),
                "bq": qkv_b[rs].reshape(FPC, 1).copy(),
                "bk": qkv_b[ks].reshape(FPC, 1).copy(),
                "bv": qkv_b[vs].reshape(FPC, 1).copy(),
                "owT": owT,
                "ob": ob,
            }
        )
    return in_maps


def assemble(results):
    full = np.empty((TOK, E), dtype=np.float32)
    for c in range(NCORES):
        full[c * TSL : (c + 1) * TSL, :] = results[c]["out"].T
    return full.reshape(B, T, E)


_NC_CACHE = None


def kernel(x, qkv_w, qkv_b, out_w, out_b):
    global _NC_CACHE
    if _NC_CACHE is None:
        _NC_CACHE = build_nc()
    in_maps = make_in_maps(x, qkv_w, qkv_b, out_w, out_b)
    res = bass_utils.run_bass_kernel_spmd(
        _NC_CACHE, in_maps, core_ids=list(range(NCORES))
    )
    return assemble(res.results)
